# revision 1
# baseline (speedup 1.0000x reference)
"""Trainium2 Bass kernel for BuildVolume2d (stereo cost volume, L1 over channels).

cost[b, d, h, w] = sum_c |feat_l[b,c,h,w] - feat_r[b,c,h,4w-d]|   (feat_r zero-padded left)

Sharding: batch B=8 -> 8 NeuronCores (data parallel, one sample per core).

Per-core layout (sample b):
  - Iterate over 64 h-groups of 4 rows. SBUF partitions = (h_in_group*32 + c);
    the host pre-transposes inputs to [(h c), w] so each group load is one flat DMA.
  - feat_r row block cast to fp16 and phase-split into Rall tile:
      Rall[:, 524*t + pad_t + j] = r[c,h,4j+t],  pad_0=11, pad_{1,2,3}=12, zeros in pads.
    For disparity d = 4q+s: shifted_r col = R_{(4-s)%4}[w - q - (s>0)], which for all
    4 phases is Rall[11 - q + 524*t + w]  (t in 0..3, d = 4q + perm[t], perm=[0,3,2,1]).
  - 12 mega subtracts per h-group (one per q): diff[128,(4t),(512w)] fp16 (DVE 2x mode).
  - |x| via relu pair: pos = max(x,0) (DVE tensor_scalar 4x), neg part either
    relu(-x) on ACT (reduced with +ones) or min(x,0) on DVE (reduced with -ones);
    the two matmuls accumulate into the same PSUM slot.
  - PSUM drained via ACT copy [96,2048] -> SBUF staging -> DMA to HBM.
"""
import sys
sys.path.insert(0, '/opt/trn_rl_repo')

import numpy as np
import concourse.bass as bass
import concourse.tile as tile
from concourse import bacc, mybir
from concourse.bass_utils import run_bass_kernel_spmd

# ---- problem constants (hardcoded per spec) ----
B, C, H, W = 8, 32, 256, 512
W4 = 4 * W
D = 48                     # maxdisp
N_CORES = 8
HG = 4                     # h rows per group
N_HG = H // HG             # 64
PW = 524                   # per-phase block width in Rall
RALL_W = 4 * PW            # 2096
RALL_ALLOC = RALL_W + 12   # slack so the q-shifted window slice stays in range
PERM = [0, 3, 2, 1]        # t -> s so that d = 4q + PERM[t]

f32 = mybir.dt.float32
fp16 = mybir.dt.float16

# engine assignment tunables (counts per h-group, out of 12 q-instructions).
# GpSimd shares an SBUF port pair with the DVE: giving it tensor work knocks
# DVE tensor_scalar from 4x to 2x mode, so it only does tiny memsets.
N_ACT_ABS = 5              # q's reduced via ACT activation(Abs) + one matmul set;
                           # the rest use the DVE relu/min pair + two matmul sets

_compiled = None


def build_program(n_hg=N_HG):
    nc = bacc.Bacc("TRN2", target_bir_lowering=False, debug=False, num_devices=N_CORES)
    # host pre-transposes to h-major rows: [(h c), w]
    fl = nc.dram_tensor("feat_l", [H * C, W], f32, kind="ExternalInput").ap()
    fr = nc.dram_tensor("feat_r", [H * C, W4], f32, kind="ExternalInput").ap()
    ones = nc.dram_tensor("ones_st", [128, 32], fp16, kind="ExternalInput").ap()
    onesn = nc.dram_tensor("ones_neg", [128, 32], fp16, kind="ExternalInput").ap()
    out = nc.dram_tensor("cost", [D, H, W], f32, kind="ExternalOutput").ap()

    with tile.TileContext(nc) as tc:
        with (
            tc.tile_pool(name="const", bufs=1) as constp,
            tc.tile_pool(name="inp", bufs=4) as inp,
            tc.tile_pool(name="r16p", bufs=3) as r16p,
            tc.tile_pool(name="l16p", bufs=3) as l16p,
            tc.tile_pool(name="diffp", bufs=6) as diffp,
            tc.tile_pool(name="absp", bufs=6) as absp,
            tc.tile_pool(name="stgp", bufs=4) as stgp,
            tc.tile_pool(name="psum", bufs=2, space="PSUM") as psp,
        ):
            ost = constp.tile([128, 32], fp16, name="ost")
            nc.sync.dma_start(ost[:], ones[:])
            ostn = constp.tile([128, 32], fp16, name="ostn")
            nc.sync.dma_start(ostn[:], onesn[:])

            def emit_loads(g):
                lf32 = inp.tile([128, W], f32, name="lf32", tag="lf32")
                nc.sync.dma_start(lf32[:], fl[128 * g:128 * (g + 1), :])
                rf32 = inp.tile([128, W4], f32, name="rf32", tag="rf32")
                nc.sync.dma_start(rf32[:], fr[128 * g:128 * (g + 1), :])

                l16 = l16p.tile([128, W], fp16, name="l16")
                nc.vector.tensor_copy(l16[:], lf32[:])

                rall = r16p.tile([128, RALL_ALLOC], fp16, name="rall")
                nc.gpsimd.memset(rall[:, 0:11], 0.0)
                nc.gpsimd.memset(rall[:, 523:536], 0.0)
                nc.gpsimd.memset(rall[:, 1047:1060], 0.0)
                nc.gpsimd.memset(rall[:, 1571:1584], 0.0)
                for t in range(4):
                    base = PW * t + (11 if t == 0 else 12)
                    src_ = rf32[:, t:W4:4]
                    dst = rall[:, base:base + W]
                    nc.scalar.copy(dst, src_)
                return l16, rall

            def emit_compute(g, l16, rall):
                h0 = HG * g
                for F in range(4):
                    pt = psp.tile([128, 2048], f32, name="pt")
                    for qi in range(3):
                        q = 3 * F + qi
                        dif = diffp.tile([128, 4, W], fp16, name="dif")
                        in0 = l16[:].unsqueeze(1).broadcast_to((128, 4, W))
                        in1 = rall[:, 11 - q: 11 - q + RALL_W] \
                            .rearrange("p (t w) -> p t w", t=4)[:, :, :W]
                        nc.vector.tensor_tensor(
                            dif[:], in0, in1, op=mybir.AluOpType.subtract)

                        d2 = dif[:].rearrange("p t w -> p (t w)")
                        if q in _ACT_ABS_QS:
                            ab = absp.tile([128, 4, W], fp16, name="ab", tag="ab")
                            nc.scalar.activation(
                                ab[:].rearrange("p t w -> p (t w)"), d2,
                                mybir.ActivationFunctionType.Abs)
                            for t in range(4):
                                fslot = PERM[t]
                                nc.tensor.matmul(
                                    pt[32 * qi:32 * qi + 32,
                                       512 * fslot:512 * fslot + 512],
                                    ost[:], ab[:, t, :], start=True, stop=True)
                        else:
                            pos = absp.tile([128, 4, W], fp16, name="pos", tag="pos")
                            nc.vector.tensor_scalar_max(
                                pos[:].rearrange("p t w -> p (t w)"), d2, 0.0)
                            neg = absp.tile([128, 4, W], fp16, name="neg", tag="neg")
                            nc.vector.tensor_scalar_min(
                                neg[:].rearrange("p t w -> p (t w)"), d2, 0.0)
                            for t in range(4):
                                fslot = PERM[t]
                                dst = pt[32 * qi:32 * qi + 32,
                                         512 * fslot:512 * fslot + 512]
                                nc.tensor.matmul(dst, ost[:], pos[:, t, :],
                                                 start=True, stop=False)
                                nc.tensor.matmul(dst, ostn[:], neg[:, t, :],
                                                 start=False, stop=True)

                    stg = stgp.tile([128, 2048], f32, name="stg")
                    nc.scalar.copy(stg[0:96, :], pt[0:96, :])
                    for b in range(3):
                        d0 = 12 * F + 4 * b
                        nc.sync.dma_start(
                            out[d0:d0 + 4, h0:h0 + HG, :].rearrange("d h w -> h d w"),
                            stg[32 * b:32 * b + 4, :].rearrange("h (d w) -> h d w", d=4))

            # 2-deep load prefetch: casts for g+1/g+2 are emitted before
            # compute of g so ACT produces rall well ahead of the DVE subs.
            q0 = emit_loads(0)
            q1 = emit_loads(1) if n_hg > 1 else None
            for g in range(n_hg):
                nxt = emit_loads(g + 2) if g + 2 < n_hg else None
                emit_compute(g, *q0)
                q0, q1 = q1, nxt
    nc.compile()
    return nc


_ACT_ABS_QS = set(q for q in range(2 * N_ACT_ABS) if q % 2 == 0)


def make_ones():
    # partition k = h*32 + c; output row m carries h == m % 4 (8 replicas so
    # every PSUM row in the 32-row group is written; DMA reads rows 0..3).
    on = np.zeros((128, 32), np.float16)
    for m in range(32):
        h = m % 4
        on[h * 32:(h + 1) * 32, m] = 1.0
    return on


def prep_in_maps(feat_l, feat_r):
    on = make_ones()
    onn = -on
    maps = []
    for i in range(N_CORES):
        flt = np.ascontiguousarray(
            feat_l[i].transpose(1, 0, 2)).reshape(H * C, W)
        frt = np.ascontiguousarray(
            feat_r[i].transpose(1, 0, 2)).reshape(H * C, W4)
        maps.append({"feat_l": flt, "feat_r": frt, "ones_st": on,
                     "ones_neg": onn})
    return maps


def kernel(feat_l, feat_r, maxdisp):
    global _compiled
    feat_l = np.asarray(feat_l, dtype=np.float32)
    feat_r = np.asarray(feat_r, dtype=np.float32)
    assert int(maxdisp) == D
    assert feat_l.shape == (B, C, H, W) and feat_r.shape == (B, C, H, W4)
    if _compiled is None:
        _compiled = build_program()
    in_maps = prep_in_maps(feat_l, feat_r)
    res = run_bass_kernel_spmd(_compiled, in_maps, list(range(N_CORES)))
    return np.stack([res.results[i]["cost"] for i in range(N_CORES)], axis=0)



# revision 19
# speedup vs baseline: 1.1670x; 1.1670x over previous
"""Trainium2 Bass kernel for BuildVolume2d (stereo cost volume, L1 over channels).

cost[b, d, h, w] = sum_c |feat_l[b,c,h,w] - feat_r[b,c,h,4w-d]|   (feat_r zero-padded left)

Sharding: batch B=8 -> 8 NeuronCores (data parallel, one sample per core).

v2 design (per core, sample b):
  - Host pre-bakes fp16 layouts so no on-chip casts / phase splits are needed:
      l16pad[(h c), 12+w]            = l[c,h,w], zero pads left/right
      rstack[(h c), 524*s + 12 + m]  = r[c,h,4m-s]  (zero where 4m-s < 0 or m < 0)
    With v = 12 + w - q the diff for d = 4q+s is
      diff[p, q, s, v] = l16pad[p, q+v] - rstack[p, 524*s + v]
    i.e. the rstack read is q-INDEPENDENT and the l read is a sliding window,
    so ALL 12*4 disparity diffs of an h-group are ONE tensor_tensor each per
    engine (DVE 2x fp16; a tail of q's on GpSimd), instead of 12 instructions.
  - |x| in place via three engines on disjoint q-ranges of the diff tile:
    ACT activation(Abs), DVE tensor_scalar(abs_max, 0) in 4x mode, GpSimd.
  - Channel reduction: one matmul per d: ones4[128,4] selects h = k//32,
    writing PSUM rows 4*(d%32)+h, col block d//32. 48 matmuls/h-group, each
    [128k x 4m x 512n] fp16 -> dense PSUM [128, 1024] f32 (only rows<64 used
    in block 1).
  - Drain: 2 ACT copies PSUM->SBUF fp16 (1024 free elems total per h-group).
  - Output DMA'd as fp16 [48,256,512]; host casts to f32.
"""
import sys
sys.path.insert(0, '/opt/trn_rl_repo')

import numpy as np
import concourse.bass as bass
import concourse.tile as tile
from concourse import bacc, mybir
from concourse.ap import AP
from concourse.bass_utils import run_bass_kernel_spmd

# ---- problem constants (hardcoded per spec) ----
B, C, H, W = 8, 32, 256, 512
W4 = 4 * W
D = 48                     # maxdisp
N_CORES = 8
HG = 4                     # h rows per group
N_HG = H // HG             # 64
NQ = 12                    # d = 4q + s
PW = 524                   # v-axis width per (q,s)
FREE_Q = 4 * PW            # 2096, free elems per q
LPAD = 12                  # left zero pad of l16pad
LW = W + 2 * LPAD          # 536 cols in l16pad

# ---- engine split knobs (tuned against TimelineSim / HW trace) ----
SQ = 9                     # DVE subtracts q in [0, SQ); GpSimd does [SQ, 12)
ABS_ACT_Q = 6              # ACT abs on q in [0, ABS_ACT_Q)
ABS_DVE_Q = 12             # DVE abs on q in [ABS_ACT_Q, ABS_DVE_Q); GpSimd rest

# debug/bisect switches (leave True for production)
EMIT_MM = True
EMIT_DRAIN = True
EMIT_OUT = True

SUB_CHUNK = 3              # DVE sub emitted in q-chunks of this size
ABS_CHUNK = 2              # ACT abs emitted in q-chunks of this size
DIFF_BUFS = 3
PSUM_BUFS = 2

f32 = mybir.dt.float32
f16 = mybir.dt.float16

_compiled = None


def _win_ap(base_ap, col0, nq):
    """Sliding-window view [128, nq, 4, PW] of a [128, LW] tile:
    col = col0 + q + v  (q-stride 1, s-stride 0, v-stride 1)."""
    part = list(base_ap.ap)[0]
    return AP(base_ap.tensor, base_ap.offset + col0,
              [part, [1, nq], [0, 4], [1, PW]])


def build_program(n_hg=N_HG):
    nc = bacc.Bacc("TRN2", target_bir_lowering=False, debug=False,
                   num_devices=N_CORES)
    fl = nc.dram_tensor("feat_l", [H * C, LW], f16, kind="ExternalInput").ap()
    fr = nc.dram_tensor("feat_r", [H * C, FREE_Q], f16,
                        kind="ExternalInput").ap()
    on = nc.dram_tensor("ones8", [128, 256], f16, kind="ExternalInput").ap()
    out = nc.dram_tensor("cost", [D, H, W], f16, kind="ExternalOutput").ap()

    sub = mybir.AluOpType.subtract
    absmax = mybir.AluOpType.abs_max

    with tile.TileContext(nc) as tc:
        with (
            tc.tile_pool(name="const", bufs=1) as constp,
            tc.tile_pool(name="inp", bufs=3) as inp,
            tc.tile_pool(name="diffp", bufs=DIFF_BUFS) as diffp,
            tc.tile_pool(name="stgp", bufs=4) as stgp,
            tc.tile_pool(name="psum", bufs=PSUM_BUFS, space="PSUM") as psp,
        ):
            # 8 one-hot stationaries: on8[j][k, m] = 1 iff m == 4*j + k//32
            on8 = constp.tile([128, 256], f16, name="on8")
            nc.sync.dma_start(on8[:], on[:])
            on8v = on8[:].rearrange("p (j m) -> p j m", j=8)

            def emit_loads(g):
                l16 = inp.tile([128, LW], f16, name="l16", tag="l16")
                nc.sync.dma_start(l16[:], fl[128 * g:128 * (g + 1), :])
                rst = inp.tile([128, FREE_Q], f16, name="rst", tag="rst")
                nc.sync.dma_start(rst[:], fr[128 * g:128 * (g + 1), :])
                return l16, rst

            def emit_compute(g, l16, rst):
                h0 = HG * g
                dif = diffp.tile([128, NQ * FREE_Q], f16, name="dif")
                dif4 = dif[:].rearrange("p (q s v) -> p q s v", q=NQ, s=4)
                rs3 = rst[:].rearrange("p (s v) -> p s v", s=4)
                la = l16[:]

                # subtracts: diff[p,q,s,v] = l16[p, q+v] - rst[p, s, v]
                # (DVE share chunked so ACT abs can start early)
                for lo in range(0, SQ, SUB_CHUNK):
                    hi = min(lo + SUB_CHUNK, SQ)
                    nc.vector.tensor_tensor(
                        dif4[:, lo:hi], _win_ap(la, lo, hi - lo),
                        rs3.unsqueeze(1).broadcast_to((128, hi - lo, 4, PW)),
                        op=sub)
                if SQ < NQ:
                    nc.gpsimd.tensor_tensor(
                        dif4[:, SQ:NQ], _win_ap(la, SQ, NQ - SQ),
                        rs3.unsqueeze(1).broadcast_to((128, NQ - SQ, 4, PW)),
                        op=sub)

                # |diff| in place, split across engines by q-range. The ACT
                # share is chunked so the PE can start on low q's early.
                dfl = dif[:]
                a0 = int(ABS_ACT_Q * FREE_Q)
                a1 = int(ABS_DVE_Q * FREE_Q)
                bounds = [q * FREE_Q
                          for q in range(0, int(ABS_ACT_Q), ABS_CHUNK)] + [a0]
                for lo, hi in zip(bounds, bounds[1:]):
                    nc.scalar.activation(dfl[:, lo:hi], dfl[:, lo:hi],
                                         mybir.ActivationFunctionType.Abs)
                # DVE abs: clear the fp16 sign bit (abs_max fails the ISA
                # check; bitwise_and on an int16 view keeps the 4x mode)
                if ABS_DVE_Q > ABS_ACT_Q:
                    dvi = dfl[:, a0:a1].bitcast(mybir.dt.int16)
                    nc.vector.tensor_scalar(dvi, dvi, 0x7fff, None,
                                            op0=mybir.AluOpType.bitwise_and)
                if ABS_DVE_Q < NQ:
                    pvi = dfl[:, a1:].bitcast(mybir.dt.int16)
                    nc.gpsimd.tensor_scalar(pvi, pvi, 0x7fff, None,
                                            op0=mybir.AluOpType.bitwise_and)

                # channel reduce: one matmul per disparity. d = 24*cb + dd,
                # PSUM rows 4*dd + h (8 matmuls accumulate per 32-row block).
                if not EMIT_MM:
                    return
                pt = psp.tile([128, 1024], f32, name="pt")
                for d_ in range(D):
                    cb, dd = d_ // 24, d_ % 24
                    blk, j = dd // 8, dd % 8
                    q, s = d_ // 4, d_ % 4
                    nc.tensor.matmul(
                        pt[32 * blk:32 * blk + 32, 512 * cb:512 * cb + 512],
                        on8v[:, j, :], dif4[:, q, s, LPAD - q:LPAD - q + W],
                        start=(j == 0), stop=(j == 7))

                # drain PSUM -> SBUF fp16
                if not EMIT_DRAIN:
                    return
                stg = stgp.tile([128, 1024], f16, name="stg")
                nc.scalar.copy(stg[0:96, :], pt[0:96, :])

                # out DMA: stg row 4*dd + h, col block cb -> out[24cb+dd, h0+h]
                if not EMIT_OUT:
                    return
                for cb in range(2):
                    nc.sync.dma_start(
                        out[24 * cb:24 * cb + 24, h0:h0 + HG, :],
                        stg[0:96, 512 * cb:512 * cb + 512])

            q0 = emit_loads(0)
            q1 = emit_loads(1) if n_hg > 1 else None
            for g in range(n_hg):
                nxt = emit_loads(g + 2) if g + 2 < n_hg else None
                emit_compute(g, *q0)
                q0, q1 = q1, nxt
    nc.compile()
    return nc


def prep_in_maps(feat_l, feat_r):
    on = np.zeros((128, 8, 32), np.float16)
    for k in range(128):
        for j in range(8):
            on[k, j, 4 * j + k // 32] = 1.0
    on = on.reshape(128, 256)

    lt = np.ascontiguousarray(feat_l.transpose(0, 2, 1, 3)) \
        .reshape(B, H * C, W).astype(np.float16)
    lp = np.zeros((B, H * C, LW), np.float16)
    lp[:, :, LPAD:LPAD + W] = lt

    rt = np.ascontiguousarray(feat_r.transpose(0, 2, 1, 3)) \
        .reshape(B, H * C, W4).astype(np.float16)
    rs = np.zeros((B, H * C, FREE_Q), np.float16)
    # col 524*s + 12 + m = r[4m - s]; valid when m >= 1, or (m == 0 and s == 0)
    rs[:, :, 12:12 + W] = rt[:, :, 0::4]                      # s = 0
    for s in (1, 2, 3):
        vals = rt[:, :, 4 - s::4][:, :, :W - 1]               # m = 1..511
        rs[:, :, 524 * s + 13:524 * s + 13 + (W - 1)] = vals

    maps = []
    for i in range(N_CORES):
        maps.append({"feat_l": lp[i], "feat_r": rs[i], "ones8": on})
    return maps


def kernel(feat_l, feat_r, maxdisp):
    global _compiled
    feat_l = np.asarray(feat_l, dtype=np.float32)
    feat_r = np.asarray(feat_r, dtype=np.float32)
    assert int(maxdisp) == D
    assert feat_l.shape == (B, C, H, W) and feat_r.shape == (B, C, H, W4)
    if _compiled is None:
        _compiled = build_program()
    in_maps = prep_in_maps(feat_l, feat_r)
    res = run_bass_kernel_spmd(_compiled, in_maps, list(range(N_CORES)))
    return np.stack(
        [res.results[i]["cost"].astype(np.float32) for i in range(N_CORES)],
        axis=0)


# revision 20
# speedup vs baseline: 1.1709x; 1.0033x over previous
"""Trainium2 Bass kernel for BuildVolume2d (stereo cost volume, L1 over channels).

cost[b, d, h, w] = sum_c |feat_l[b,c,h,w] - feat_r[b,c,h,4w-d]|   (feat_r zero-padded left)

Sharding: batch B=8 -> 8 NeuronCores (data parallel, one sample per core).

v3 design (per core, sample b; h-groups of 4 rows, partitions p = 32h + c):
  - Host pre-bakes fp16 layouts (no on-chip casts / phase splits):
      l16[(h c), w]                  = l[c,h,w]
      rstack[(h c), 524*s + 12 + m]  = r[c,h,4m-s]  (zero where 4m-s < 0 or m < 0)
    For d = 4q+s:  diff[p, q, s, w] = l16[p, w] - rstack[p, 524*s + 12 + w - q]
    i.e. in0 is a pure broadcast over (q, s) and in1 has q-stride -1: big
    multi-q subtracts are single instructions with a dense [128, n*4*512] out.
  - |x| in place, split across engines by q-range. Engine split tuned against
    the HW trace: the chip power-throttles (~0.7 util) and SBUF ports are
    contended, so ACT (1x but otherwise idle) carries most of the abs;
    DVE (4x tensor_scalar via int16 sign-bit clear) takes a little; GpSimd
    subtracts the tail q's.
  - Channel reduce: one matmul per disparity d, moving = dif[:, 512d:512d+512],
    8 one-hot stationaries on8[j][k, 4j + k//32]; 8 matmuls accumulate per
    32-row PSUM block; PSUM rows 4*dd + h, col block cb (d = 24cb + dd).
  - Drain: ACT copy PSUM[96,1024] f32 -> SBUF fp16.
  - Output DMA'd as fp16 [48,256,512] (2D sbuf -> 3D dram AP); host casts f32.
"""
import sys
sys.path.insert(0, '/opt/trn_rl_repo')

import numpy as np
import concourse.bass as bass
import concourse.tile as tile
from concourse import bacc, mybir
from concourse.ap import AP
from concourse.bass_utils import run_bass_kernel_spmd

# ---- problem constants (hardcoded per spec) ----
B, C, H, W = 8, 32, 256, 512
W4 = 4 * W
D = 48                     # maxdisp
N_CORES = 8
HG = 4                     # h rows per group
N_HG = H // HG             # 64
NQ = 12                    # d = 4q + s
RW = 2096                  # rstack row width (4 phase blocks of 524)
FQ = 4 * W                 # 2048 diff elems per q

# ---- engine split knobs (tuned against HW traces) ----
SQ = 8                     # DVE subtracts q in [0, SQ); GpSimd does [SQ, 12)
ABS_ACT_Q = 11             # ACT abs on q in [0, ABS_ACT_Q)
ABS_DVE_Q = 12             # DVE abs on [ABS_ACT_Q, ABS_DVE_Q); GpSimd rest
SUB_CHUNK = 3              # DVE sub emitted in q-chunks of this size
ABS_CHUNK = 2              # ACT abs emitted in q-chunks of this size
DIFF_BUFS = 3
PSUM_BUFS = 2

f32 = mybir.dt.float32
f16 = mybir.dt.float16
i16 = mybir.dt.int16

_compiled = None


def build_program(n_hg=N_HG):
    nc = bacc.Bacc("TRN2", target_bir_lowering=False, debug=False,
                   num_devices=N_CORES)
    fl = nc.dram_tensor("feat_l", [H * C, W], f16, kind="ExternalInput").ap()
    fr = nc.dram_tensor("feat_r", [H * C, RW], f16,
                        kind="ExternalInput").ap()
    on = nc.dram_tensor("ones8", [128, 256], f16, kind="ExternalInput").ap()
    out = nc.dram_tensor("cost", [D, H, W], f16, kind="ExternalOutput").ap()

    sub = mybir.AluOpType.subtract
    band = mybir.AluOpType.bitwise_and

    with tile.TileContext(nc) as tc:
        with (
            tc.tile_pool(name="const", bufs=1) as constp,
            tc.tile_pool(name="inp", bufs=3) as inp,
            tc.tile_pool(name="diffp", bufs=DIFF_BUFS) as diffp,
            tc.tile_pool(name="stgp", bufs=4) as stgp,
            tc.tile_pool(name="psum", bufs=PSUM_BUFS, space="PSUM") as psp,
        ):
            # 8 one-hot stationaries: on8[j][k, m] = 1 iff m == 4*j + k//32
            on8 = constp.tile([128, 256], f16, name="on8")
            nc.sync.dma_start(on8[:], on[:])
            on8v = on8[:].rearrange("p (j m) -> p j m", j=8)

            def emit_loads(g):
                l16 = inp.tile([128, W], f16, name="l16", tag="l16")
                nc.sync.dma_start(l16[:], fl[128 * g:128 * (g + 1), :])
                rst = inp.tile([128, RW], f16, name="rst", tag="rst")
                nc.sync.dma_start(rst[:], fr[128 * g:128 * (g + 1), :])
                return l16, rst

            def sub_aps(la, ra, lo, hi):
                n = hi - lo
                lpart = list(la.ap)[0]
                rpart = list(ra.ap)[0]
                in0 = AP(la.tensor, la.offset,
                         [lpart, [0, n], [0, 4], [1, W]])
                in1 = AP(ra.tensor, ra.offset + 12 - lo,
                         [rpart, [-1, n], [524, 4], [1, W]])
                return in0, in1

            def emit_compute(g, l16, rst):
                h0 = HG * g
                dif = diffp.tile([128, NQ * FQ], f16, name="dif")
                dif4 = dif[:].rearrange("p (q s w) -> p q s w", q=NQ, s=4)
                la, ra = l16[:], rst[:]

                # subtracts: DVE q in [0, SQ) chunked; GpSimd the rest
                for lo in range(0, SQ, SUB_CHUNK):
                    hi = min(lo + SUB_CHUNK, SQ)
                    in0, in1 = sub_aps(la, ra, lo, hi)
                    nc.vector.tensor_tensor(dif4[:, lo:hi], in0, in1, op=sub)
                if SQ < NQ:
                    in0, in1 = sub_aps(la, ra, SQ, NQ)
                    nc.gpsimd.tensor_tensor(dif4[:, SQ:NQ], in0, in1, op=sub)

                # |diff| in place, split by q-range; ACT chunked so the PE
                # can start on low q's early
                dfl = dif[:]
                a0, a1 = ABS_ACT_Q * FQ, ABS_DVE_Q * FQ
                bounds = [q * FQ
                          for q in range(0, ABS_ACT_Q, ABS_CHUNK)] + [a0]
                for lo, hi in zip(bounds, bounds[1:]):
                    nc.scalar.activation(dfl[:, lo:hi], dfl[:, lo:hi],
                                         mybir.ActivationFunctionType.Abs)
                # DVE/GpSimd abs: clear fp16 sign bit on an int16 view
                # (abs_max fails the walrus ISA check; this keeps 4x mode)
                if ABS_DVE_Q > ABS_ACT_Q:
                    dvi = dfl[:, a0:a1].bitcast(i16)
                    nc.vector.tensor_scalar(dvi, dvi, 0x7fff, None, op0=band)
                if ABS_DVE_Q < NQ:
                    pvi = dfl[:, a1:].bitcast(i16)
                    nc.gpsimd.tensor_scalar(pvi, pvi, 0x7fff, None, op0=band)

                # channel reduce: one matmul per disparity. d = 24*cb + dd,
                # PSUM rows 4*dd + h (8 matmuls accumulate per 32-row block).
                pt = psp.tile([128, 1024], f32, name="pt")
                for d_ in range(D):
                    cb, dd = d_ // 24, d_ % 24
                    blk, j = dd // 8, dd % 8
                    nc.tensor.matmul(
                        pt[32 * blk:32 * blk + 32, 512 * cb:512 * cb + 512],
                        on8v[:, j, :], dfl[:, W * d_:W * d_ + W],
                        start=(j == 0), stop=(j == 7))

                # drain PSUM -> SBUF fp16
                stg = stgp.tile([128, 1024], f16, name="stg")
                nc.scalar.copy(stg[0:96, :], pt[0:96, :])

                # out DMA: stg row 4*dd + h, col block cb -> out[24cb+dd, h0+h]
                for cb in range(2):
                    nc.sync.dma_start(
                        out[24 * cb:24 * cb + 24, h0:h0 + HG, :],
                        stg[0:96, 512 * cb:512 * cb + 512])

            q0 = emit_loads(0)
            q1 = emit_loads(1) if n_hg > 1 else None
            for g in range(n_hg):
                nxt = emit_loads(g + 2) if g + 2 < n_hg else None
                emit_compute(g, *q0)
                q0, q1 = q1, nxt
    nc.compile()
    return nc


def prep_in_maps(feat_l, feat_r):
    on = np.zeros((128, 8, 32), np.float16)
    for k in range(128):
        for j in range(8):
            on[k, j, 4 * j + k // 32] = 1.0
    on = on.reshape(128, 256)

    lt = np.ascontiguousarray(feat_l.transpose(0, 2, 1, 3)) \
        .reshape(B, H * C, W).astype(np.float16)

    rt = np.ascontiguousarray(feat_r.transpose(0, 2, 1, 3)) \
        .reshape(B, H * C, W4).astype(np.float16)
    rs = np.zeros((B, H * C, RW), np.float16)
    # col 524*s + 12 + m = r[4m - s]; valid when m >= 1, or (m == 0 and s == 0)
    rs[:, :, 12:12 + W] = rt[:, :, 0::4]                      # s = 0
    for s in (1, 2, 3):
        vals = rt[:, :, 4 - s::4][:, :, :W - 1]               # m = 1..511
        rs[:, :, 524 * s + 13:524 * s + 13 + (W - 1)] = vals

    maps = []
    for i in range(N_CORES):
        maps.append({"feat_l": lt[i], "feat_r": rs[i], "ones8": on})
    return maps


def kernel(feat_l, feat_r, maxdisp):
    global _compiled
    feat_l = np.asarray(feat_l, dtype=np.float32)
    feat_r = np.asarray(feat_r, dtype=np.float32)
    assert int(maxdisp) == D
    assert feat_l.shape == (B, C, H, W) and feat_r.shape == (B, C, H, W4)
    if _compiled is None:
        _compiled = build_program()
    in_maps = prep_in_maps(feat_l, feat_r)
    res = run_bass_kernel_spmd(_compiled, in_maps, list(range(N_CORES)))
    return np.stack(
        [res.results[i]["cost"].astype(np.float32) for i in range(N_CORES)],
        axis=0)
